# revision 5
# baseline (speedup 1.0000x reference)
"""Trainium2 Bass kernel for a dense pre-LN transformer block.

B=4, T=1024, C=1024, H=16 heads (head_size 64).

Distribution over the 8 NeuronCores (two SPMD launches, host-side
reduction between them):

  Launch A (attention, head-parallel): every core runs the identical
  program on all 4 batches but with its own pair of heads (weight
  slices are per-core input data). Each core produces the partial
  out @ Wo contribution of its 2 heads for the whole [B*T, C] output.
  NOTE the reference computes scores as k @ q^T (roles of q/k swapped
  vs standard attention) — handled by using k rows as the "queries".

  Host: x2 = x + sum_c partial_c + bo.

  Launch B (FFN, row-parallel): core c runs LN2 + W1/PReLU/W2 + residual
  on rows [512c, 512(c+1)) of x2.

Matmuls run in float32r (full PE rate at N>=256); the BIR verifier
requires every matmul operand to be *produced* as float32r, so all
matmul-feeding tiles/DRAM tensors are declared float32r (bit-identical
to fp32 in numpy terms; HW rounds on write).
"""

import os
from contextlib import ExitStack

import numpy as np

import concourse.bass as bass
import concourse.tile as tile
from concourse import bacc, mybir
from concourse.bass_utils import run_bass_kernel_spmd
from concourse.masks import make_identity, make_causal_mask

F32 = mybir.dt.float32
F32R = mybir.dt.float32r
BF16 = mybir.dt.bfloat16
# FFN W1/W2/fT/h2T dtype: BF16 halves the dominant 32MB weight stream
# (rel-err impact validated on HW before adoption)
FFN_WDT = BF16
AF = mybir.ActivationFunctionType
ALU = mybir.AluOpType

B, T, C, H, HS = 4, 1024, 1024, 16, 64
NCORES = 8
EPS = 1e-5
SCALE = float(C) ** -0.5  # 1/32, folded into the softmax exp
NEG = -1e30

NTB = T // 128   # 8 token blocks per batch
NCC = C // 128   # 8 channel chunks


# --------------------------------------------------------------------------
# kernel A: attention, 2 heads per core, all batches
# --------------------------------------------------------------------------

def _attn_body(ctx, tc, x, wq, wk, wv, lnw, lnb, catout):
    """Transposed-scores attention: scoresT[s,t] with s on partitions.

    softmax denominator comes from an appended ones-column in v (av psum
    column 64), normalization is a per-partition scale on the av output,
    so no wei transposes are needed; only [t,d]->[d,t] cat transposes.
    """
    nc = tc.nc

    const = ctx.enter_context(tc.tile_pool(name="const", bufs=1))
    scratch = const.tile([128, 128], F32)
    make_identity(nc, scratch)
    ident = const.tile([128, 128], F32R)
    nc.vector.tensor_copy(out=ident, in_=scratch)
    # transposed causal mask for diagonal blocks: keep s<=t (cols>=rows)
    trilT = const.tile([128, 128], F32)
    nc.gpsimd.memset(trilT, 0.0)
    nc.gpsimd.affine_select(
        out=trilT, in_=trilT, compare_op=ALU.is_ge, fill=NEG, base=0,
        pattern=[[1, 128]], channel_multiplier=-1)
    ones8 = const.tile([128, NTB], F32)
    nc.vector.memset(ones8, 1.0)
    zero132 = const.tile([128, 132], F32)
    nc.vector.memset(zero132, 0.0)
    eps_t = const.tile([128, 1], F32)
    nc.vector.memset(eps_t, EPS)

    wq_sb = const.tile([128, NCC, 128], F32R, tag="wq")
    wk_sb = const.tile([128, NCC, 128], F32R, tag="wk")
    wv_sb = const.tile([128, NCC, 128], F32R, tag="wv")
    nc.sync.dma_start(out=wq_sb, in_=wq.rearrange("(cc p) d -> p cc d", p=128))
    nc.sync.dma_start(out=wk_sb, in_=wk.rearrange("(cc p) d -> p cc d", p=128))
    nc.sync.dma_start(out=wv_sb, in_=wv.rearrange("(cc p) d -> p cc d", p=128))
    general_ln = lnw is not None
    if general_ln:
        lnw_bc = const.tile([128, C], F32, tag="lnw")
        lnb_bc = const.tile([128, C], F32, tag="lnb")
        nc.sync.dma_start(
            out=lnw_bc,
            in_=bass.AP(tensor=lnw.tensor, offset=lnw.offset,
                        ap=[[0, 128]] + list(lnw.ap)))
        nc.sync.dma_start(
            out=lnb_bc,
            in_=bass.AP(tensor=lnb.tensor, offset=lnb.offset,
                        ap=[[0, 128]] + list(lnb.ap)))

    xp = ctx.enter_context(tc.tile_pool(name="xp", bufs=5))
    hp = ctx.enter_context(tc.tile_pool(name="hp", bufs=9))
    hTp = ctx.enter_context(tc.tile_pool(name="hTp", bufs=1))
    stat = ctx.enter_context(tc.tile_pool(name="stat", bufs=4))
    qkp = ctx.enter_context(tc.tile_pool(name="qkp", bufs=2))
    vp = ctx.enter_context(tc.tile_pool(name="vp", bufs=2))
    epl = ctx.enter_context(tc.tile_pool(name="epl", bufs=2))
    ctkp = ctx.enter_context(tc.tile_pool(name="ctkp", bufs=10))

    # PSUM banks: mm 2x[128,512]=2, score 2x[128,1024]=4, tr4 2x[128,512]=2
    PSM = ctx.enter_context(tc.tile_pool(name="psm", bufs=2, space="PSUM"))
    PSS = ctx.enter_context(tc.tile_pool(name="pss", bufs=2, space="PSUM"))
    PST = ctx.enter_context(tc.tile_pool(name="pst", bufs=2, space="PSUM"))

    for b in range(B):
        # ---- LN1: rstd batched per group of 4 token tiles ----
        h_tiles = []
        for g in range(2):
            mvs = stat.tile([128, 4, 2], F32, tag="mvs", name=f"mvs_{b}_{g}")
            rstd = stat.tile([128, 4], F32, tag="rstd", name=f"rstd_{b}_{g}")
            lnv = stat.tile([128, 4], F32, tag="lnv", name=f"lnv_{b}_{g}")
            xts = []
            for j in range(4):
                i = g * 4 + j
                r0 = (b * NTB + i) * 128
                xt = xp.tile([128, C], F32, tag="x", name=f"x_{b}_{i}")
                nc.sync.dma_start(out=xt, in_=x[r0:r0 + 128, :])
                st = stat.tile([128, 2, 6], F32, tag="bn", name=f"bn_{b}_{i}")
                for k in range(2):
                    nc.vector.bn_stats(out=st[:, k, :],
                                       in_=xt[:, k * 512:(k + 1) * 512])
                nc.vector.bn_aggr(out=mvs[:, j, :], in_=st)
                xts.append(xt)
            nc.scalar.activation(out=lnv, in_=mvs[:, :, 1], func=AF.Ln,
                                 bias=eps_t)
            nc.scalar.activation(out=rstd, in_=lnv, func=AF.Exp, scale=-0.5)
            for j in range(4):
                i = g * 4 + j
                ht = hp.tile([128, C], F32R, tag="h", name=f"h_{b}_{i}")
                nc.gpsimd.tensor_scalar(
                    out=ht, in0=xts[j], scalar1=mvs[:, j, 0:1],
                    scalar2=rstd[:, j:j + 1], op0=ALU.subtract, op1=ALU.mult)
                if general_ln:
                    nc.vector.tensor_mul(out=ht, in0=ht, in1=lnw_bc)
                    nc.vector.tensor_add(out=ht, in0=ht, in1=lnb_bc)
                h_tiles.append(ht)

        # ---- transpose h -> hT, grouped 4 blocks per psum/copy ----
        hT = hTp.tile([128, NCC, T], F32R, tag="hT")
        for cc in range(NCC):
            for g in range(2):
                pt = PST.tile([128, 512], F32R, tag="tr4",
                              name=f"pt_{b}_{cc}_{g}")
                for j in range(4):
                    i = g * 4 + j
                    nc.tensor.transpose(
                        pt[:, j * 128:(j + 1) * 128],
                        h_tiles[i][:, cc * 128:(cc + 1) * 128], ident)
                eng = nc.vector if (cc + g) % 2 else nc.scalar
                if eng is nc.scalar:
                    nc.scalar.copy(
                        out=hT[:, cc, g * 512:(g + 1) * 512], in_=pt)
                else:
                    nc.vector.tensor_copy(
                        out=hT[:, cc, g * 512:(g + 1) * 512], in_=pt)

        # ---- qkv (2 heads packed: d2 = 128) ----
        qT2 = qkp.tile([128, T], F32R, tag="qT", name=f"qT_{b}")
        kT2 = qkp.tile([128, T], F32R, tag="kT", name=f"kT_{b}")
        for tch in range(T // 512):
            tsl = slice(tch * 512, (tch + 1) * 512)
            pq = PSM.tile([128, 512], F32, tag="mm", name=f"pq_{b}_{tch}")
            for cc in range(NCC):
                nc.tensor.matmul(pq, wq_sb[:, cc, :], hT[:, cc, tsl],
                                 start=(cc == 0), stop=(cc == NCC - 1))
            nc.scalar.copy(out=qT2[:, tsl], in_=pq)
            pk = PSM.tile([128, 512], F32, tag="mm", name=f"pk_{b}_{tch}")
            for cc in range(NCC):
                nc.tensor.matmul(pk, wk_sb[:, cc, :], hT[:, cc, tsl],
                                 start=(cc == 0), stop=(cc == NCC - 1))
            nc.scalar.copy(out=kT2[:, tsl], in_=pk)
        # v2: [t_part, sc, 130]: per head 65 cols (64 v + ones)
        v2 = vp.tile([128, NTB, 132], F32R, tag="v2", name=f"v2_{b}")
        for i in range(NTB):
            nc.vector.tensor_copy(out=v2[:, i, :], in_=zero132)
            nc.vector.tensor_copy(out=v2[:, i, 64:65], in_=ones8[:, i:i + 1])
            nc.vector.tensor_copy(out=v2[:, i, 130:131], in_=ones8[:, i:i + 1])
        for i in range(NTB):
            pv = PSM.tile([128, 128], F32, tag="mm", name=f"pv_{b}_{i}")
            for cc in range(NCC):
                nc.tensor.matmul(pv, hT[:, cc, i * 128:(i + 1) * 128],
                                 wv_sb[:, cc, :],
                                 start=(cc == 0), stop=(cc == NCC - 1))
            nc.vector.tensor_copy(out=v2[:, i, 0:64], in_=pv[:, 0:64])
            nc.vector.tensor_copy(out=v2[:, i, 66:130], in_=pv[:, 64:128])

        # ---- attention ----
        cat_toks = [ctkp.tile([128, 128], F32R, tag="ctk",
                              name=f"ctk_{b}_{i}") for i in range(NTB)]
        for h in range(2):
            hsl = slice(h * 64, (h + 1) * 64)
            # scoresT + exp, one psum + one exp per s-chunk
            eps_list = []
            for sc in range(NTB):
                W = (NTB - sc) * 128  # t columns: blocks sc..7
                pss = PSS.tile([128, W], F32, tag="score",
                               name=f"pss_{b}_{h}_{sc}")
                for j in range(NTB - sc):
                    i = sc + j
                    nc.tensor.matmul(
                        pss[:, j * 128:(j + 1) * 128],
                        qT2[hsl, sc * 128:(sc + 1) * 128],
                        kT2[hsl, i * 128:(i + 1) * 128],
                        start=True, stop=True)
                nc.vector.tensor_add(out=pss[:, 0:128], in0=pss[:, 0:128],
                                     in1=trilT)
                e_sc = epl.tile([128, W], F32R, tag=f"e{sc}",
                                name=f"e_{b}_{h}_{sc}")
                # one exp op per PSUM bank (bank-crossing ACT reads are
                # suspect for the NRT_EXEC_UNIT_UNRECOVERABLE wedge)
                n0 = 0
                while n0 < W:
                    n1 = min(n0 + 512, W)
                    nc.scalar.activation(out=e_sc[:, n0:n1],
                                         in_=pss[:, n0:n1], func=AF.Exp,
                                         scale=SCALE)
                    n0 = n1
                eps_list.append(e_sc)
            # av + normalize into cat_tok
            for i in range(NTB):
                po = PSM.tile([128, 66], F32, tag="mm",
                              name=f"po_{b}_{h}_{i}")
                for sc in range(i + 1):
                    j = i - sc
                    nc.tensor.matmul(
                        po, eps_list[sc][:, j * 128:(j + 1) * 128],
                        v2[:, sc, h * 66:(h + 1) * 66],
                        start=(sc == 0), stop=(sc == i))
                rec = stat.tile([128, 1], F32, tag="rec",
                                name=f"rec_{b}_{h}_{i}")
                nc.vector.reciprocal(out=rec, in_=po[:, 64:65])
                nc.vector.tensor_scalar_mul(
                    out=cat_toks[i][:, hsl], in0=po[:, 0:64], scalar1=rec)

        # ---- write per-head outputs straight to DRAM ----
        for i in range(NTB):
            r0 = (b * NTB + i) * 128
            nc.sync.dma_start(out=catout[r0:r0 + 128, :], in_=cat_toks[i])


def _build_attn(general_ln: bool, repeat: int = 1):
    nc = bacc.Bacc("TRN2", target_bir_lowering=False, debug=False)
    x = nc.dram_tensor("x", [B * T, C], F32, kind="ExternalInput").ap()
    wq = nc.dram_tensor("wq", [C, 128], F32R, kind="ExternalInput").ap()
    wk = nc.dram_tensor("wk", [C, 128], F32R, kind="ExternalInput").ap()
    wv = nc.dram_tensor("wv", [C, 128], F32R, kind="ExternalInput").ap()
    lnw = lnb = None
    if general_ln:
        lnw = nc.dram_tensor("lnw", [C], F32, kind="ExternalInput").ap()
        lnb = nc.dram_tensor("lnb", [C], F32, kind="ExternalInput").ap()
    catout = nc.dram_tensor("catout", [B * T, 128], F32R,
                            kind="ExternalOutput").ap()
    with tile.TileContext(nc) as tc:
        for _ in range(repeat):
            with ExitStack() as ctx:
                _attn_body(ctx, tc, x, wq, wk, wv, lnw, lnb, catout)
    nc.compile()
    return nc


# --------------------------------------------------------------------------
# kernel B: FFN, 512 rows per core
# --------------------------------------------------------------------------

RPC = (B * T) // NCORES  # 512 rows per core
NRB = RPC // 128         # 4 row blocks
NHID = 4 * C // 128      # 32 hidden chunks


def _ffn_body(ctx, tc, xr, cat, wo, w1, w2, bo, b1, ln2w, ln2b, b2,
              alpha, out, wdt=F32R):
    """Per-core rows: proj = cat @ Wo (+bo); x2 = x + proj; LN2 + FFN.

    wdt: dtype for W1/W2/fT/h2T (F32R, or BF16 to halve weight DMA).
    Wo/cat stay F32R (projection feeds the residual directly).
    """
    nc = tc.nc
    general_ln = ln2w is not None

    const = ctx.enter_context(tc.tile_pool(name="const", bufs=1))
    scratch = const.tile([128, 128], F32)
    make_identity(nc, scratch)
    ident = const.tile([128, 128], F32R)
    nc.vector.tensor_copy(out=ident, in_=scratch)
    eps_t = const.tile([128, 1], F32)
    nc.vector.memset(eps_t, EPS)
    b1_sb = None
    if b1 is not None:
        b1_sb = const.tile([128, NHID], F32, tag="b1")
        nc.sync.dma_start(out=b1_sb, in_=b1.rearrange("(h p) -> p h", p=128))

    def bcast(src, tag):
        t = const.tile([128, C], F32, tag=tag, name=tag)
        nc.sync.dma_start(
            out=t, in_=bass.AP(tensor=src.tensor, offset=src.offset,
                               ap=[[0, 128]] + list(src.ap)))
        return t

    bo_bc = bcast(bo, "bo") if bo is not None else None
    lnw_bc = bcast(ln2w, "lnw") if general_ln else None
    lnb_bc = bcast(ln2b, "lnb") if general_ln else None
    b2_bc = bcast(b2, "b2") if b2 is not None else None

    xrp = ctx.enter_context(tc.tile_pool(name="xrp", bufs=2))
    catp = ctx.enter_context(tc.tile_pool(name="catp", bufs=2))
    x2p = ctx.enter_context(tc.tile_pool(name="x2p", bufs=NRB))
    hp = ctx.enter_context(tc.tile_pool(name="hp", bufs=2))
    cTp = ctx.enter_context(tc.tile_pool(name="cTp", bufs=1))
    h2Tp = ctx.enter_context(tc.tile_pool(name="h2Tp", bufs=1))
    stat = ctx.enter_context(tc.tile_pool(name="stat", bufs=8))
    wop = ctx.enter_context(tc.tile_pool(name="wop", bufs=3))
    w1p = ctx.enter_context(tc.tile_pool(name="w1p", bufs=3))
    w2p = ctx.enter_context(tc.tile_pool(name="w2p", bufs=3))
    ftp = ctx.enter_context(tc.tile_pool(name="ftp", bufs=NHID))
    tmp = ctx.enter_context(tc.tile_pool(name="tmp", bufs=3))
    osb = ctx.enter_context(tc.tile_pool(name="osb", bufs=2))

    x2_tiles = []
    # ---- cat rows -> catT ----
    catT = cTp.tile([128, NCC, RPC], F32R, tag="catT")
    with tc.tile_pool(name="pst0", bufs=2, space="PSUM") as PST0:
        for r in range(NRB):
            ct = catp.tile([128, C], F32R, tag="cat", name=f"cat_{r}")
            nc.sync.dma_start(out=ct, in_=cat[r * 128:(r + 1) * 128, :])
            for cc in range(NCC):
                pt = PST0.tile([128, 128], F32R, tag="tr",
                               name=f"ptc_{r}_{cc}")
                nc.tensor.transpose(
                    pt, ct[:, cc * 128:(cc + 1) * 128], ident)
                nc.scalar.copy(out=catT[:, cc, r * 128:(r + 1) * 128],
                               in_=pt)

    # ---- proj (Wo streamed per cc) + residual -> x2 ----
    with tc.tile_pool(name="psp", bufs=NRB, space="PSUM") as PSP:
        pps = [PSP.tile([128, C], F32, tag="pp", name=f"pp_{r}")
               for r in range(NRB)]
        for cc in range(NCC):
            wo_sb = wop.tile([128, C], F32R, tag="wo", name=f"wo_{cc}")
            nc.sync.dma_start(out=wo_sb, in_=wo[cc * 128:(cc + 1) * 128, :])
            for r in range(NRB):
                for co in range(2):
                    csl = slice(co * 512, (co + 1) * 512)
                    nc.tensor.matmul(pps[r][:, csl],
                                     catT[:, cc, r * 128:(r + 1) * 128],
                                     wo_sb[:, csl],
                                     start=(cc == 0), stop=(cc == NCC - 1))
        for r in range(NRB):
            xt = xrp.tile([128, C], F32, tag="xr", name=f"xr_{r}")
            nc.sync.dma_start(out=xt, in_=xr[r * 128:(r + 1) * 128, :])
            x2t = x2p.tile([128, C], F32, tag="x2", name=f"x2_{r}")
            nc.vector.tensor_add(out=x2t, in0=pps[r], in1=xt)
            if bo_bc is not None:
                nc.vector.tensor_add(out=x2t, in0=x2t, in1=bo_bc)
            x2_tiles.append(x2t)

    h2T = h2Tp.tile([128, NCC, RPC], wdt, tag="h2T")
    with tc.tile_pool(name="pst", bufs=2, space="PSUM") as PST, \
         tc.tile_pool(name="psf", bufs=2, space="PSUM") as PSF:
        # ---- LN2 + transpose ----
        for r in range(NRB):
            xt = x2_tiles[r]
            st = stat.tile([128, 2, 6], F32, tag="bn", name=f"bn_{r}")
            for k in range(2):
                nc.vector.bn_stats(out=st[:, k, :],
                                   in_=xt[:, k * 512:(k + 1) * 512])
            mv = stat.tile([128, 2], F32, tag="mv", name=f"mv_{r}")
            nc.vector.bn_aggr(out=mv, in_=st)
            lnv = stat.tile([128, 1], F32, tag="lnv", name=f"lnv_{r}")
            nc.scalar.activation(out=lnv, in_=mv[:, 1:2], func=AF.Ln,
                                 bias=eps_t)
            rstd = stat.tile([128, 1], F32, tag="rstd", name=f"rstd_{r}")
            nc.scalar.activation(out=rstd, in_=lnv, func=AF.Exp, scale=-0.5)
            ht = hp.tile([128, C], F32R, tag="h", name=f"h_{r}")
            nc.gpsimd.tensor_scalar(
                out=ht, in0=xt, scalar1=mv[:, 0:1], scalar2=rstd,
                op0=ALU.subtract, op1=ALU.mult)
            if general_ln:
                nc.vector.tensor_mul(out=ht, in0=ht, in1=lnw_bc)
                nc.vector.tensor_add(out=ht, in0=ht, in1=lnb_bc)
            for cc in range(NCC):
                pt = PST.tile([128, 128], F32R, tag="tr4",
                              name=f"pt_{r}_{cc}")
                nc.tensor.transpose(pt, ht[:, cc * 128:(cc + 1) * 128], ident)
                nc.scalar.copy(out=h2T[:, cc, r * 128:(r + 1) * 128], in_=pt)

        # ---- phase 1: fT[h] = prelu(W1_h^T @ h2 + b1) ----
        f_tiles = []
        w1r = w1.rearrange("(cc p) (h q) -> p cc h q", p=128, q=128)
        for h in range(NHID):
            w1_sb = w1p.tile([128, NCC, 128], wdt, tag="w1",
                             name=f"w1_{h}")
            nc.sync.dma_start(out=w1_sb, in_=w1r[:, :, h, :])
            pf = PSF.tile([128, RPC], F32, tag="ft", name=f"pf_{h}")
            for cc in range(NCC):
                nc.tensor.matmul(pf, w1_sb[:, cc, :], h2T[:, cc, :],
                                 start=(cc == 0), stop=(cc == NCC - 1))
            ft = ftp.tile([128, RPC], wdt, tag="ft", name=f"ft_{h}")
            if b1_sb is not None:
                src = tmp.tile([128, RPC], F32, tag="pb", name=f"pb_{h}")
                nc.vector.tensor_scalar_add(out=src, in0=pf,
                                            scalar1=b1_sb[:, h:h + 1])
            else:
                src = pf
            tneg = tmp.tile([128, RPC], F32, tag="tneg", name=f"tneg_{h}")
            nc.vector.tensor_scalar(
                out=tneg, in0=src, scalar1=0.0, scalar2=alpha - 1.0,
                op0=ALU.min, op1=ALU.mult)
            nc.vector.tensor_add(out=ft, in0=src, in1=tneg)
            f_tiles.append(ft)

    # ---- phase 2: out = fT.T @ W2 (+b2) + x2 ----
    with tc.tile_pool(name="pso", bufs=NRB, space="PSUM") as PSO:
        pouts = [PSO.tile([128, C], F32, tag="out", name=f"pout{r}")
                 for r in range(NRB)]
        for h in range(NHID):
            w2_sb = w2p.tile([128, C], wdt, tag="w2", name=f"w2_{h}")
            nc.sync.dma_start(out=w2_sb, in_=w2[h * 128:(h + 1) * 128, :])
            for r in range(NRB):
                for co in range(2):
                    csl = slice(co * 512, (co + 1) * 512)
                    nc.tensor.matmul(pouts[r][:, csl],
                                     f_tiles[h][:, r * 128:(r + 1) * 128],
                                     w2_sb[:, csl],
                                     start=(h == 0), stop=(h == NHID - 1))
        for r in range(NRB):
            o_sb = osb.tile([128, C], F32, tag="o", name=f"o_{r}")
            nc.vector.tensor_add(out=o_sb, in0=pouts[r], in1=x2_tiles[r])
            if b2_bc is not None:
                nc.vector.tensor_add(out=o_sb, in0=o_sb, in1=b2_bc)
            nc.sync.dma_start(out=out[r * 128:(r + 1) * 128, :], in_=o_sb)


def _build_ffn(general_ln: bool, has_bo: bool, has_b1: bool, has_b2: bool,
               alpha: float, repeat: int = 1, wdt=F32R):
    nc = bacc.Bacc("TRN2", target_bir_lowering=False, debug=False)
    xr = nc.dram_tensor("xr", [RPC, C], F32, kind="ExternalInput").ap()
    cat = nc.dram_tensor("cat", [RPC, C], F32R, kind="ExternalInput").ap()
    wo = nc.dram_tensor("wo", [C, C], F32R, kind="ExternalInput").ap()
    w1 = nc.dram_tensor("w1", [C, 4 * C], wdt, kind="ExternalInput").ap()
    w2 = nc.dram_tensor("w2", [4 * C, C], wdt, kind="ExternalInput").ap()
    bo = b1 = ln2w = ln2b = b2 = None
    if has_bo:
        bo = nc.dram_tensor("bo", [C], F32, kind="ExternalInput").ap()
    if has_b1:
        b1 = nc.dram_tensor("b1", [4 * C], F32, kind="ExternalInput").ap()
    if general_ln:
        ln2w = nc.dram_tensor("ln2w", [C], F32, kind="ExternalInput").ap()
        ln2b = nc.dram_tensor("ln2b", [C], F32, kind="ExternalInput").ap()
    if has_b2:
        b2 = nc.dram_tensor("b2", [C], F32, kind="ExternalInput").ap()
    out = nc.dram_tensor("out", [RPC, C], F32, kind="ExternalOutput").ap()
    with tile.TileContext(nc) as tc:
        for _ in range(repeat):
            with ExitStack() as ctx:
                _ffn_body(ctx, tc, xr, cat, wo, w1, w2, bo, b1, ln2w, ln2b, b2,
                          alpha, out, wdt=wdt)
    nc.compile()
    return nc


# --------------------------------------------------------------------------
# host orchestration
# --------------------------------------------------------------------------

_NC_CACHE = {}

# bench-only instrumentation: when KBENCH_TRACE is set, launches run with
# trace=True and per-launch device exec_time_ns is appended here.
_TRACE = bool(os.environ.get("KBENCH_TRACE"))
EXEC_NS = []
TRACE_PATHS = []


def _run_spmd(nc, in_maps):
    res = run_bass_kernel_spmd(nc, in_maps, list(range(NCORES)),
                               trace=_TRACE,
                               trace_cores=list(range(NCORES)) if _TRACE
                               else None)
    if _TRACE:
        EXEC_NS.append(res.exec_time_ns)
        if res.instructions_and_trace is not None:
            TRACE_PATHS.append(res.instructions_and_trace[1])
    return res


def _get_attn_nc(general_ln):
    key = ("attn", general_ln)
    if key not in _NC_CACHE:
        _NC_CACHE[key] = _build_attn(general_ln)
    return _NC_CACHE[key]


def _get_ffn_nc(general_ln, has_bo, has_b1, has_b2, alpha, wdt=None):
    wdt = FFN_WDT if wdt is None else wdt
    key = ("ffn", general_ln, has_bo, has_b1, has_b2, float(alpha), wdt)
    if key not in _NC_CACHE:
        _NC_CACHE[key] = _build_ffn(general_ln, has_bo, has_b1, has_b2,
                                    float(alpha), wdt=wdt)
    return _NC_CACHE[key]


def _w_np(a):
    if FFN_WDT == BF16:
        import ml_dtypes
        return np.ascontiguousarray(a.astype(ml_dtypes.bfloat16))
    return a


def attn_in_maps(x_flat, Wq, Wk, Wv, trivial, ln1_w, ln1_b):
    in_maps = []
    for c in range(NCORES):
        h0 = 2 * c
        m = {
            "x": x_flat,
            "wq": np.ascontiguousarray(
                np.concatenate([Wq[h0], Wq[h0 + 1]], axis=1)),
            "wk": np.ascontiguousarray(
                np.concatenate([Wk[h0], Wk[h0 + 1]], axis=1)),
            "wv": np.ascontiguousarray(
                np.concatenate([Wv[h0], Wv[h0 + 1]], axis=1)),
        }
        if not trivial:
            m["lnw"] = ln1_w
            m["lnb"] = ln1_b
        in_maps.append(m)
    return in_maps


def run_attn(x_flat, Wq, Wk, Wv, ln1_w, ln1_b):
    """Returns cat [B*T, C]: per-head attention outputs, head-major cols."""
    trivial = bool(np.all(ln1_w == 1.0) and np.all(ln1_b == 0.0))
    nc = _get_attn_nc(not trivial)
    in_maps = attn_in_maps(x_flat, Wq, Wk, Wv, trivial, ln1_w, ln1_b)
    res = _run_spmd(nc, in_maps)
    return np.concatenate(
        [res.results[c]["catout"] for c in range(NCORES)], axis=1)


def ffn_in_maps(x_flat, cat_all, Wo, bo, W1, b1, W2, b2, ln2_w, ln2_b,
                flags):
    trivial, has_bo, has_b1, has_b2 = flags
    in_maps = []
    for c in range(NCORES):
        sl = slice(RPC * c, RPC * (c + 1))
        m = {
            "xr": np.ascontiguousarray(x_flat[sl]),
            "cat": np.ascontiguousarray(cat_all[sl]),
            "wo": Wo,
            "w1": _w_np(W1),
            "w2": _w_np(W2),
        }
        if has_bo:
            m["bo"] = bo
        if has_b1:
            m["b1"] = b1
        if not trivial:
            m["ln2w"] = ln2_w
            m["ln2b"] = ln2_b
        if has_b2:
            m["b2"] = b2
        in_maps.append(m)
    return in_maps


def run_ffn(x_flat, cat_all, Wo, bo, W1, b1, W2, b2, ln2_w, ln2_b, alpha):
    trivial = bool(np.all(ln2_w == 1.0) and np.all(ln2_b == 0.0))
    has_bo = bool(np.any(bo != 0.0))
    has_b1 = bool(np.any(b1 != 0.0))
    has_b2 = bool(np.any(b2 != 0.0))
    nc = _get_ffn_nc(not trivial, has_bo, has_b1, has_b2, alpha)
    flags = (trivial, has_bo, has_b1, has_b2)
    in_maps = ffn_in_maps(x_flat, cat_all, Wo, bo, W1, b1, W2, b2,
                          ln2_w, ln2_b, flags)
    res = _run_spmd(nc, in_maps)
    return np.concatenate(
        [res.results[c]["out"] for c in range(NCORES)], axis=0)


def kernel(x, ln1_w, ln1_b, Wk, Wq, Wv, Wo, bo, ln2_w, ln2_b, W1, b1,
           prelu_a, W2, b2):
    x = np.asarray(x, np.float32)
    x_flat = np.ascontiguousarray(x.reshape(B * T, C))
    Wq = np.asarray(Wq, np.float32)
    Wk = np.asarray(Wk, np.float32)
    Wv = np.asarray(Wv, np.float32)
    Wo = np.asarray(Wo, np.float32)
    alpha = float(np.asarray(prelu_a))

    cat_all = run_attn(x_flat, Wq, Wk, Wv,
                       np.asarray(ln1_w, np.float32),
                       np.asarray(ln1_b, np.float32))
    out = run_ffn(x_flat, cat_all, Wo, np.asarray(bo, np.float32),
                  np.asarray(W1, np.float32), np.asarray(b1, np.float32),
                  np.asarray(W2, np.float32), np.asarray(b2, np.float32),
                  np.asarray(ln2_w, np.float32),
                  np.asarray(ln2_b, np.float32), alpha)
    return out.reshape(B, T, C).astype(np.float32)



# revision 33
# speedup vs baseline: 2.6855x; 2.6855x over previous
"""Trainium2 Bass kernel for a dense pre-LN transformer block.

B=4, T=1024, C=1024, H=16 heads (head_size 64).

Distribution over 8 NeuronCores, two SPMD launches with a free host-side
reduction between them:

  Launch A (attention): core c works on batch b=c//2 and head-half
  hh=c%2 (8 heads). It computes LN1 for its batch only, projects
  q/k/v for its heads, runs causal softmax(k@q^T)-attention in the
  transposed-scores layout, and multiplies by its slice of Wo rows,
  producing a PARTIAL projection [T, C] (f32) for its batch.

  Host: x2[b] = x[b] + part[2b] + part[2b+1] (+bo).

  Launch B (FFN): core c runs LN2 + W1/PReLU/W2 + residual on rows
  [512c, 512(c+1)) of x2.

Matmul dtype strategy: the PE cost depends only on the MOVING operand
dtype and its free size (1 cycle/row for bf16 at any N, f32r at N>=256).
Activations that move (hT, wo, w2 stream, e) stay f32r/bf16 chosen for
SBUF fit; weights that sit stationary are bf16 (0.4% quantization).
Accumulation is always f32 in PSUM.
"""

import os
from contextlib import ExitStack

import numpy as np

import concourse.bass as bass
import concourse.tile as tile
from concourse import bacc, mybir
from concourse.bass_utils import run_bass_kernel_spmd
from concourse.masks import make_identity

F32 = mybir.dt.float32
F32R = mybir.dt.float32r
BF16 = mybir.dt.bfloat16
AF = mybir.ActivationFunctionType
ALU = mybir.AluOpType

B, T, C, H, HS = 4, 1024, 1024, 16, 64
NCORES = 8
EPS = 1e-5
SCALE = float(C) ** -0.5  # folded into the softmax exp
NEG = -1e30

NTB = T // 128   # 8 token blocks per batch
NCC = C // 128   # 8 channel chunks
HPC = H // 2     # 8 heads per core
RPC = (B * T) // NCORES  # 512 rows per core in launch B
NRB = RPC // 128         # 4 row blocks
NHID = 4 * C // 128      # 32 hidden chunks


def _bank_chunks(lo, hi):
    """Split [lo, hi) at 512-column PSUM bank boundaries."""
    out = []
    o = lo
    while o < hi:
        n = min(512 - (o % 512), hi - o)
        out.append((o, o + n))
        o += n
    return out


# --------------------------------------------------------------------------
# kernel A: attention, one batch + 8 heads per core
# --------------------------------------------------------------------------

def _attn_body(ctx, tc, x, wq, wk, wv, wo, lnw, lnb, ones_dram, pout):
    nc = tc.nc
    general_ln = lnw is not None

    const = ctx.enter_context(tc.tile_pool(name="const", bufs=1))
    hTp = ctx.enter_context(tc.tile_pool(name="hTp", bufs=1))
    qTp = ctx.enter_context(tc.tile_pool(name="qTp", bufs=4))
    kTp = ctx.enter_context(tc.tile_pool(name="kTp", bufs=4))
    v2p = ctx.enter_context(tc.tile_pool(name="v2p", bufs=1))
    stat = ctx.enter_context(tc.tile_pool(name="stat", bufs=2))
    ep = ctx.enter_context(tc.tile_pool(name="ep", bufs=2))
    avp = ctx.enter_context(tc.tile_pool(name="avp", bufs=4))
    ctp = ctx.enter_context(tc.tile_pool(name="ctp", bufs=4))
    osp = ctx.enter_context(tc.tile_pool(name="osp", bufs=2))

    xp_cm = tc.tile_pool(name="xp", bufs=8)
    hp_cm = tc.tile_pool(name="hp", bufs=3)
    xp = xp_cm.__enter__()
    hp = hp_cm.__enter__()
    # x tiles first: these DMAs gate the LN1 critical path
    xts = []
    for i in range(NTB):
        xt = xp.tile([128, C], F32, tag="x", name=f"x_{i}")
        nc.sync.dma_start(out=xt, in_=x[i * 128:(i + 1) * 128, :])
        xts.append(xt)

    scratch = const.tile([128, 128], F32)
    make_identity(nc, scratch)
    ident = const.tile([128, 128], BF16)
    nc.vector.tensor_copy(out=ident, in_=scratch)
    # transposed causal mask for diagonal blocks: keep s<=t (cols>=rows)
    trilT = const.tile([128, 128], F32)
    nc.gpsimd.memset(trilT, 0.0)
    nc.gpsimd.affine_select(
        out=trilT, in_=trilT, compare_op=ALU.is_ge, fill=NEG, base=0,
        pattern=[[1, 128]], channel_multiplier=-1)
    eps_t = const.tile([128, 1], F32)
    nc.vector.memset(eps_t, EPS)
    ones64 = const.tile([1, 64], BF16)
    nc.sync.dma_start(out=ones64, in_=ones_dram)
    if general_ln:
        lnw_bc = const.tile([128, C], F32, tag="lnw")
        lnb_bc = const.tile([128, C], F32, tag="lnb")
        nc.sync.dma_start(
            out=lnw_bc,
            in_=bass.AP(tensor=lnw.tensor, offset=lnw.offset,
                        ap=[[0, 128]] + list(lnw.ap)))
        nc.sync.dma_start(
            out=lnb_bc,
            in_=bass.AP(tensor=lnb.tensor, offset=lnb.offset,
                        ap=[[0, 128]] + list(lnb.ap)))

    # weights resident in SBUF (after x: LN1 must not wait behind these)
    wq_sb = const.tile([128, NCC, 512], BF16, tag="wq")
    wk_sb = const.tile([128, NCC, 512], BF16, tag="wk")
    wv_sb = const.tile([128, NCC, 512], BF16, tag="wv")
    wo_sb = const.tile([128, 4, C], BF16, tag="wo")
    nc.sync.dma_start(out=wq_sb, in_=wq)
    nc.sync.dma_start(out=wk_sb, in_=wk)
    nc.sync.dma_start(out=wv_sb, in_=wv)
    nc.sync.dma_start(out=wo_sb, in_=wo)

    hT = hTp.tile([128, NCC, T], BF16, tag="hT")

    # ---- Phase A: LN1 (own batch only) + transpose ----
    with tc.tile_pool(name="pst", bufs=2, space="PSUM") as PST:
        mvs = stat.tile([128, NTB, 2], F32, tag="mvs")
        rstd = stat.tile([128, NTB], F32, tag="rstd")
        lnv = stat.tile([128, NTB], F32, tag="lnv")
        for i in range(NTB):
            st = stat.tile([128, 2, 6], F32, tag="bn", name=f"bn_{i}")
            for k in range(2):
                nc.vector.bn_stats(out=st[:, k, :],
                                   in_=xts[i][:, k * 512:(k + 1) * 512])
            nc.vector.bn_aggr(out=mvs[:, i, :], in_=st)
        nc.scalar.activation(out=lnv, in_=mvs[:, :, 1], func=AF.Ln,
                             bias=eps_t)
        nc.scalar.activation(out=rstd, in_=lnv, func=AF.Exp, scale=-0.5)
        for i in range(NTB):
            ht = hp.tile([128, C], BF16, tag="h", name=f"h_{i}")
            nc.vector.tensor_scalar(
                out=ht, in0=xts[i], scalar1=mvs[:, i, 0:1],
                scalar2=rstd[:, i:i + 1], op0=ALU.subtract, op1=ALU.mult)
            if general_ln:
                nc.vector.tensor_mul(out=ht, in0=ht, in1=lnw_bc)
                nc.vector.tensor_add(out=ht, in0=ht, in1=lnb_bc)
            for g in range(2):
                pt = PST.tile([128, 512], BF16, tag="tr",
                              name=f"pt_{i}_{g}")
                for c in range(4):
                    cc = g * 4 + c
                    nc.tensor.transpose(
                        pt[:, c * 128:(c + 1) * 128],
                        ht[:, cc * 128:(cc + 1) * 128], ident)
                for c in range(4):
                    cc = g * 4 + c
                    nc.vector.tensor_copy(
                        out=hT[:, cc, i * 128:(i + 1) * 128],
                        in_=pt[:, c * 128:(c + 1) * 128])
    hp_cm.__exit__(None, None, None)
    xp_cm.__exit__(None, None, None)
    dnp = ctx.enter_context(tc.tile_pool(name="dnp", bufs=8))
    lnp = ctx.enter_context(tc.tile_pool(name="lnp", bufs=4))
    recp = ctx.enter_context(tc.tile_pool(name="recp", bufs=8))

    # ---- Phase B+C+D interleaved: qkv, scores+exp, av ----
    qT = [qTp.tile([128, T], BF16, tag="qT", name=f"qT_{m}")
          for m in range(4)]
    kT = [kTp.tile([128, T], BF16, tag="kT", name=f"kT_{m}")
          for m in range(4)]
    # v2[tok, sc, h, 0:64] = v; col 64 = ones (softmax denominator)
    v2 = v2p.tile([128, NTB, HPC, 66], BF16, tag="v2")
    nc.vector.memset(v2[:, :, :, 64:65], 1.0)
    nc.vector.memset(v2[:, :, :, 65:66], 0.0)
    cat = [ctp.tile([128, T], BF16, tag="cat", name=f"cat_{m}")
           for m in range(4)]
    # per-head softmax denominator rows and their reciprocals
    dens = [dnp.tile([1, T], BF16, tag="den", name=f"den_{h}")
            for h in range(HPC)]
    recs = [recp.tile([1, T], BF16, tag="rec", name=f"rec_{h}")
            for h in range(HPC)]
    pav_sbs = [avp.tile([128, T], F32R, tag="avsb", name=f"avsb_{m}")
               for m in range(4)]

    # PSUM budget: PSQ 1x[128,512]=1 bank, PSS 3x[128,<=512]=3,
    # PSA 2x[66,1024]=4  -> 8 banks total.
    with tc.tile_pool(name="psq", bufs=1, space="PSUM") as PSQ, \
         tc.tile_pool(name="pss", bufs=3, space="PSUM") as PSS, \
         tc.tile_pool(name="psa", bufs=2, space="PSUM") as PSA:

        def qkv_m(m):
            for th in range(2):
                tsl = slice(th * 512, (th + 1) * 512)
                pq = PSQ.tile([128, 512], F32, tag="mm", name=f"pq_{m}_{th}")
                for cc in range(NCC):
                    nc.tensor.matmul(pq, wq_sb[:, cc, m * 128:(m + 1) * 128],
                                     hT[:, cc, tsl],
                                     start=(cc == 0), stop=(cc == NCC - 1))
                nc.vector.tensor_copy(out=qT[m][:, tsl], in_=pq)
                pk = PSQ.tile([128, 512], F32, tag="mm", name=f"pk_{m}_{th}")
                for cc in range(NCC):
                    nc.tensor.matmul(pk, wk_sb[:, cc, m * 128:(m + 1) * 128],
                                     hT[:, cc, tsl],
                                     start=(cc == 0), stop=(cc == NCC - 1))
                nc.vector.tensor_copy(out=kT[m][:, tsl], in_=pk)

        def v_tb(tb):
            pv = PSQ.tile([128, 512], F32, tag="mm", name=f"pv_{tb}")
            for cc in range(NCC):
                nc.tensor.matmul(pv, hT[:, cc, tb * 128:(tb + 1) * 128],
                                 wv_sb[:, cc, :],
                                 start=(cc == 0), stop=(cc == NCC - 1))
            nc.vector.tensor_copy(
                out=v2[:, tb, :, 0:64],
                in_=bass.AP(tensor=pv.tensor, offset=pv.offset,
                            ap=list(pv.ap[:1]) + [[64, HPC], [1, 64]]))

        def scores_h(h):
            """scoresT + exp for head h; returns e tiles per sc."""
            m, ho = h // 2, (h % 2) * 64
            qh = qT[m][ho:ho + 64, :]
            kh = kT[m][ho:ho + 64, :]
            es = []
            for sc in range(NTB):
                W = T - sc * 128
                e = ep.tile([128, W], BF16, tag=f"e{sc}", name=f"e_{h}_{sc}")
                for (o0, o1) in _bank_chunks(0, W):
                    ps = PSS.tile([128, o1 - o0], F32, tag="sc",
                                  name=f"ps_{h}_{sc}_{o0}")
                    nc.tensor.matmul(
                        ps,
                        qh[:, sc * 128:(sc + 1) * 128],
                        kh[:, sc * 128 + o0:sc * 128 + o1],
                        start=True, stop=True)
                    if o0 == 0:
                        nc.vector.tensor_add(out=ps[:, 0:128],
                                             in0=ps[:, 0:128], in1=trilT)
                    nc.scalar.activation(out=e[:, o0:o1], in_=ps,
                                         func=AF.Exp, scale=SCALE)
                es.append(e)
            return es

        def av_h(h, es):
            """av for head h into pav_sb half + stash denominator row."""
            pav = PSA.tile([66, T], F32, tag="av", name=f"pav_{h}")
            for sc in range(NTB):
                lo = sc * 128
                for (o0, o1) in _bank_chunks(lo, T):
                    nc.tensor.matmul(
                        pav[:, o0:o1], v2[:, sc, h, :],
                        es[sc][:, o0 - lo:o1 - lo],
                        start=(sc == 0), stop=(sc == NTB - 1),
                        skip_group_check=True)
            ho = (h % 2) * 64
            nc.vector.tensor_copy(out=pav_sbs[h // 2][ho:ho + 64, :],
                                  in_=pav[0:64, :])
            nc.scalar.copy(out=dens[h], in_=pav[64:65, :])

        es0 = qkv_m(0) or scores_h(0)
        qkv_m(1)
        for tb in range(NTB):
            v_tb(tb)
        es1 = scores_h(1)
        qkv_m(2)
        es2 = scores_h(2)
        qkv_m(3)

        es = {0: es0, 1: es1, 2: es2}
        for h in range(HPC):
            if h + 3 < HPC:
                es[h + 3] = scores_h(h + 3)
            av_h(h, es.pop(h))

    # ---- Phase E: 1/den via exp(-ln(den)), normalize, Wo projection ----
    with tc.tile_pool(name="psb", bufs=2, space="PSUM") as PSB, \
         tc.tile_pool(name="psp", bufs=2, space="PSUM") as PSP:
        lnds = [lnp.tile([1, T], F32, tag="lnd", name=f"lnd_{i}")
                for i in range(4)]
        with nc.allow_low_precision(reason="softmax denominator recip"):
            for grp in range(2):
                for i in range(4):
                    nc.scalar.activation(out=lnds[i], in_=dens[grp * 4 + i],
                                         func=AF.Ln)
                for i in range(4):
                    nc.scalar.activation(out=recs[grp * 4 + i], in_=lnds[i],
                                         func=AF.Exp, scale=-1.0)
        for m in range(4):
            # partition-broadcast each head's 1/den row via K=1 PE matmul
            prec = PSB.tile([128, T], F32, tag="prec", name=f"prec_{m}")
            for hh in range(2):
                for co in range(2):
                    nc.tensor.matmul(
                        prec[hh * 64:(hh + 1) * 64, co * 512:(co + 1) * 512],
                        ones64,
                        recs[2 * m + hh][:, co * 512:(co + 1) * 512],
                        start=True, stop=True)
            nc.vector.tensor_mul(out=cat[m], in0=pav_sbs[m], in1=prec)

        for tb in range(NTB):
            pp = PSP.tile([128, C], F32, tag="pp", name=f"pp_{tb}")
            for m in range(4):
                for co in range(2):
                    nc.tensor.matmul(
                        pp[:, co * 512:(co + 1) * 512],
                        cat[m][:, tb * 128:(tb + 1) * 128],
                        wo_sb[:, m, co * 512:(co + 1) * 512],
                        start=(m == 0), stop=(m == 3))
            o_sb = osp.tile([128, C], F32, tag="o", name=f"o_{tb}")
            nc.vector.tensor_copy(out=o_sb, in_=pp)
            nc.sync.dma_start(out=pout[tb * 128:(tb + 1) * 128, :], in_=o_sb)


def _build_attn(general_ln: bool):
    nc = bacc.Bacc("TRN2", target_bir_lowering=False, debug=False)
    x = nc.dram_tensor("x", [T, C], F32, kind="ExternalInput").ap()
    wq = nc.dram_tensor("wq", [128, NCC, 512], BF16, kind="ExternalInput").ap()
    wk = nc.dram_tensor("wk", [128, NCC, 512], BF16, kind="ExternalInput").ap()
    wv = nc.dram_tensor("wv", [128, NCC, 512], BF16, kind="ExternalInput").ap()
    wo = nc.dram_tensor("wo", [128, 4, C], BF16, kind="ExternalInput").ap()
    lnw = lnb = None
    if general_ln:
        lnw = nc.dram_tensor("lnw", [C], F32, kind="ExternalInput").ap()
        lnb = nc.dram_tensor("lnb", [C], F32, kind="ExternalInput").ap()
    ones_dram = nc.dram_tensor("ones", [1, 64], BF16,
                               kind="ExternalInput").ap()
    pout = nc.dram_tensor("pout", [T, C], F32, kind="ExternalOutput").ap()
    with tile.TileContext(nc) as tc:
        with ExitStack() as ctx:
            _attn_body(ctx, tc, x, wq, wk, wv, wo, lnw, lnb, ones_dram, pout)
    nc.compile()
    return nc


# --------------------------------------------------------------------------
# kernel B: FFN, 512 rows per core
# --------------------------------------------------------------------------

def _ffn_body(ctx, tc, x2, w1, w2, b1, lnw, lnb, alpha, out):
    nc = tc.nc
    general_ln = lnw is not None

    const = ctx.enter_context(tc.tile_pool(name="const", bufs=1))
    xp = ctx.enter_context(tc.tile_pool(name="xp", bufs=NRB))
    # x2 tiles first: these DMAs gate the LN2 critical path
    x2ts = []
    for r in range(NRB):
        xt = xp.tile([128, C], F32, tag="x", name=f"x_{r}")
        nc.sync.dma_start(out=xt, in_=x2[r * 128:(r + 1) * 128, :])
        x2ts.append(xt)
    scratch = const.tile([128, 128], F32)
    make_identity(nc, scratch)
    ident = const.tile([128, 128], BF16)
    nc.vector.tensor_copy(out=ident, in_=scratch)
    eps_t = const.tile([128, 1], F32)
    nc.vector.memset(eps_t, EPS)
    if general_ln:
        lnw_bc = const.tile([128, C], F32, tag="lnw")
        lnb_bc = const.tile([128, C], F32, tag="lnb")
        nc.sync.dma_start(
            out=lnw_bc,
            in_=bass.AP(tensor=lnw.tensor, offset=lnw.offset,
                        ap=[[0, 128]] + list(lnw.ap)))
        nc.sync.dma_start(
            out=lnb_bc,
            in_=bass.AP(tensor=lnb.tensor, offset=lnb.offset,
                        ap=[[0, 128]] + list(lnb.ap)))
    b1_sb = None
    if b1 is not None:
        b1_sb = const.tile([128, NHID], F32, tag="b1")
        nc.sync.dma_start(out=b1_sb, in_=b1.rearrange("(h p) -> p h", p=128))

    # weights: big resident tiles, streamed in chunks of 8 hidden blocks
    w1_sb = const.tile([128, NHID, NCC, 128], BF16, tag="w1")
    w2_sb = const.tile([128, NHID, C], BF16, tag="w2")
    for hg in range(4):
        hsl = slice(hg * 8, (hg + 1) * 8)
        nc.sync.dma_start(out=w1_sb[:, hsl, :, :], in_=w1[:, hsl, :, :])
    for hg in range(4):
        hsl = slice(hg * 8, (hg + 1) * 8)
        nc.sync.dma_start(out=w2_sb[:, hsl, :], in_=w2[:, hsl, :])

    hp = ctx.enter_context(tc.tile_pool(name="hp", bufs=5))
    hTp = ctx.enter_context(tc.tile_pool(name="hTp", bufs=1))
    stat = ctx.enter_context(tc.tile_pool(name="stat", bufs=2))
    ftp = ctx.enter_context(tc.tile_pool(name="ftp", bufs=NHID))
    tmp = ctx.enter_context(tc.tile_pool(name="tmp", bufs=2))
    osp = ctx.enter_context(tc.tile_pool(name="osp", bufs=2))

    h2T = hTp.tile([128, NCC, RPC], BF16, tag="h2T")

    # ---- LN2 + transpose ----
    with tc.tile_pool(name="pst", bufs=2, space="PSUM") as PST:
        mvs = stat.tile([128, NRB, 2], F32, tag="mvs")
        rstd = stat.tile([128, NRB], F32, tag="rstd")
        lnv = stat.tile([128, NRB], F32, tag="lnv")
        for r in range(NRB):
            st = stat.tile([128, 2, 6], F32, tag="bn", name=f"bn_{r}")
            for k in range(2):
                nc.vector.bn_stats(out=st[:, k, :],
                                   in_=x2ts[r][:, k * 512:(k + 1) * 512])
            nc.vector.bn_aggr(out=mvs[:, r, :], in_=st)
        nc.scalar.activation(out=lnv, in_=mvs[:, :, 1], func=AF.Ln,
                             bias=eps_t)
        nc.scalar.activation(out=rstd, in_=lnv, func=AF.Exp, scale=-0.5)
        hts = []
        for r in range(NRB):
            ht = hp.tile([128, C], BF16, tag="h", name=f"h_{r}")
            nc.vector.tensor_scalar(
                out=ht, in0=x2ts[r], scalar1=mvs[:, r, 0:1],
                scalar2=rstd[:, r:r + 1], op0=ALU.subtract, op1=ALU.mult)
            if general_ln:
                nc.vector.tensor_mul(out=ht, in0=ht, in1=lnw_bc)
                nc.vector.tensor_add(out=ht, in0=ht, in1=lnb_bc)
            hts.append(ht)
        for cc in range(NCC):
            pt = PST.tile([128, RPC], BF16, tag="tr", name=f"pt_{cc}")
            for r in range(NRB):
                nc.tensor.transpose(
                    pt[:, r * 128:(r + 1) * 128],
                    hts[r][:, cc * 128:(cc + 1) * 128], ident)
            nc.vector.tensor_copy(out=h2T[:, cc, :], in_=pt)

    # ---- W1 + PReLU ----
    f_tiles = []
    with tc.tile_pool(name="psf", bufs=2, space="PSUM") as PSF:
        for h in range(NHID):
            pf = PSF.tile([128, RPC], F32, tag="f", name=f"pf_{h}")
            for cc in range(NCC):
                nc.tensor.matmul(pf, w1_sb[:, h, cc, :], h2T[:, cc, :],
                                 start=(cc == 0), stop=(cc == NCC - 1))
            if b1_sb is not None:
                nc.vector.tensor_scalar_add(out=pf, in0=pf,
                                            scalar1=b1_sb[:, h:h + 1])
            t1 = tmp.tile([128, RPC], F32, tag="t1", name=f"t1_{h}")
            nc.vector.tensor_scalar(
                out=t1, in0=pf, scalar1=0.0, scalar2=alpha - 1.0,
                op0=ALU.min, op1=ALU.mult)
            ft = ftp.tile([128, RPC], BF16, tag="ft", name=f"ft_{h}")
            nc.vector.tensor_add(out=ft, in0=pf, in1=t1)
            f_tiles.append(ft)

    # ---- W2 + residual ----
    with tc.tile_pool(name="pso", bufs=2, space="PSUM") as PSO:
        for tb in range(NRB):
            po = PSO.tile([128, C], F32, tag="o", name=f"po_{tb}")
            for h in range(NHID):
                for co in range(2):
                    nc.tensor.matmul(
                        po[:, co * 512:(co + 1) * 512],
                        f_tiles[h][:, tb * 128:(tb + 1) * 128],
                        w2_sb[:, h, co * 512:(co + 1) * 512],
                        start=(h == 0), stop=(h == NHID - 1))
            o_sb = osp.tile([128, C], F32, tag="osb", name=f"osb_{tb}")
            nc.vector.tensor_add(out=o_sb, in0=po, in1=x2ts[tb])
            nc.sync.dma_start(out=out[tb * 128:(tb + 1) * 128, :], in_=o_sb)


def _build_ffn(general_ln: bool, has_b1: bool, alpha: float):
    nc = bacc.Bacc("TRN2", target_bir_lowering=False, debug=False)
    x2 = nc.dram_tensor("x2", [RPC, C], F32, kind="ExternalInput").ap()
    w1 = nc.dram_tensor("w1", [128, NHID, NCC, 128], BF16,
                        kind="ExternalInput").ap()
    w2 = nc.dram_tensor("w2", [128, NHID, C], BF16,
                        kind="ExternalInput").ap()
    b1 = lnw = lnb = None
    if has_b1:
        b1 = nc.dram_tensor("b1", [4 * C], F32, kind="ExternalInput").ap()
    if general_ln:
        lnw = nc.dram_tensor("lnw", [C], F32, kind="ExternalInput").ap()
        lnb = nc.dram_tensor("lnb", [C], F32, kind="ExternalInput").ap()
    out = nc.dram_tensor("out", [RPC, C], F32, kind="ExternalOutput").ap()
    with tile.TileContext(nc) as tc:
        with ExitStack() as ctx:
            _ffn_body(ctx, tc, x2, w1, w2, b1, lnw, lnb, alpha, out)
    nc.compile()
    return nc


# --------------------------------------------------------------------------
# host orchestration
# --------------------------------------------------------------------------

_NC_CACHE = {}

# bench-only instrumentation: when KBENCH_TRACE is set, launches run with
# trace=True and per-launch device exec_time_ns is appended here.
_TRACE = bool(os.environ.get("KBENCH_TRACE"))
EXEC_NS = []
TRACE_PATHS = []


def _run_spmd(nc, in_maps):
    res = run_bass_kernel_spmd(nc, in_maps, list(range(NCORES)),
                               trace=_TRACE,
                               trace_cores=list(range(NCORES)) if _TRACE
                               else None)
    if _TRACE:
        EXEC_NS.append(res.exec_time_ns)
        if res.instructions_and_trace is not None:
            TRACE_PATHS.append(res.instructions_and_trace[1])
    return res


def _bf16(a):
    import ml_dtypes
    return np.ascontiguousarray(np.asarray(a, np.float32)
                                .astype(ml_dtypes.bfloat16))


def _get_attn_nc(general_ln):
    key = ("attn", general_ln)
    if key not in _NC_CACHE:
        _NC_CACHE[key] = _build_attn(general_ln)
    return _NC_CACHE[key]


def _get_ffn_nc(general_ln, has_b1, alpha):
    key = ("ffn", general_ln, has_b1, float(alpha))
    if key not in _NC_CACHE:
        _NC_CACHE[key] = _build_ffn(general_ln, has_b1, float(alpha))
    return _NC_CACHE[key]


def _attn_weights(Wq, Wk, Wv, Wo):
    """Per-core weight arrays in the device layouts."""
    per_core = []
    for c in range(NCORES):
        hh = c % 2
        h0 = HPC * hh
        # [C, 512] -> [128, NCC, 512]
        def wlay(Wx):
            catw = np.concatenate([Wx[h] for h in range(h0, h0 + HPC)],
                                  axis=1)  # [C, 512]
            return _bf16(catw.reshape(NCC, 128, 512).transpose(1, 0, 2))
        wo = _bf16(Wo[hh * 512:(hh + 1) * 512].reshape(4, 128, C)
                   .transpose(1, 0, 2))
        per_core.append((wlay(Wq), wlay(Wk), wlay(Wv), wo))
    return per_core


def run_attn(x_flat, Wq, Wk, Wv, Wo, ln1_w, ln1_b):
    """Returns list of per-core partial projections [T, C] f32."""
    trivial = bool(np.all(ln1_w == 1.0) and np.all(ln1_b == 0.0))
    nc = _get_attn_nc(not trivial)
    wts = _attn_weights(Wq, Wk, Wv, Wo)
    in_maps = []
    for c in range(NCORES):
        b = c // 2
        wq, wk, wv, wo = wts[c]
        import ml_dtypes
        m = {"x": np.ascontiguousarray(x_flat[b * T:(b + 1) * T]),
             "wq": wq, "wk": wk, "wv": wv, "wo": wo,
             "ones": np.ones((1, 64), ml_dtypes.bfloat16)}
        if not trivial:
            m["lnw"] = np.asarray(ln1_w, np.float32)
            m["lnb"] = np.asarray(ln1_b, np.float32)
        in_maps.append(m)
    res = _run_spmd(nc, in_maps)
    return [res.results[c]["pout"] for c in range(NCORES)]


def run_ffn(x2_flat, W1, b1, W2, ln2_w, ln2_b, alpha):
    trivial = bool(np.all(ln2_w == 1.0) and np.all(ln2_b == 0.0))
    has_b1 = bool(np.any(b1 != 0.0))
    nc = _get_ffn_nc(not trivial, has_b1, alpha)
    w1l = _bf16(np.asarray(W1, np.float32)
                .reshape(NCC, 128, NHID, 128).transpose(1, 2, 0, 3))
    w2l = _bf16(np.asarray(W2, np.float32)
                .reshape(NHID, 128, C).transpose(1, 0, 2))
    in_maps = []
    for c in range(NCORES):
        m = {"x2": np.ascontiguousarray(x2_flat[RPC * c:RPC * (c + 1)]),
             "w1": w1l, "w2": w2l}
        if has_b1:
            m["b1"] = np.asarray(b1, np.float32)
        if not trivial:
            m["lnw"] = np.asarray(ln2_w, np.float32)
            m["lnb"] = np.asarray(ln2_b, np.float32)
        in_maps.append(m)
    res = _run_spmd(nc, in_maps)
    return np.concatenate(
        [res.results[c]["out"] for c in range(NCORES)], axis=0)


def kernel(x, ln1_w, ln1_b, Wk, Wq, Wv, Wo, bo, ln2_w, ln2_b, W1, b1,
           prelu_a, W2, b2):
    x = np.asarray(x, np.float32)
    x_flat = np.ascontiguousarray(x.reshape(B * T, C))
    alpha = float(np.asarray(prelu_a))

    parts = run_attn(x_flat, np.asarray(Wq, np.float32),
                     np.asarray(Wk, np.float32),
                     np.asarray(Wv, np.float32),
                     np.asarray(Wo, np.float32),
                     np.asarray(ln1_w, np.float32),
                     np.asarray(ln1_b, np.float32))
    # host reduction: x2 = x + partial_even + partial_odd (+ bo)
    x2 = np.empty_like(x_flat)
    for b in range(B):
        x2[b * T:(b + 1) * T] = (x_flat[b * T:(b + 1) * T]
                                 + parts[2 * b] + parts[2 * b + 1])
    bo = np.asarray(bo, np.float32)
    if np.any(bo != 0.0):
        x2 += bo
    out = run_ffn(x2, W1, np.asarray(b1, np.float32), W2,
                  np.asarray(ln2_w, np.float32),
                  np.asarray(ln2_b, np.float32), alpha)
    b2 = np.asarray(b2, np.float32)
    if np.any(b2 != 0.0):
        out = out + b2
    return out.reshape(B, T, C).astype(np.float32)
